# revision 2
# baseline (speedup 1.0000x reference)
"""Trainium2 Bass kernel for fused QKV linear + multi-adapter LoRA (moe_routing).

Reference computation (all fp32):
    base = x @ W^T + bias                      x:[B,S,D]  W:[3D,D]
    tmp[p,n,b,s,r]  = x . lora_A[p,n,r,:]      (down-projection, rank 16)
    tmp *= scaling[n] * lora_masks[n,b]
    lora[p,b,s,o]   = tmp . lora_B[p,n,o,r]    (up-projection, summed over n)
    out = base + concat_p(lora)                [B,S,3D]

Sharding: row-parallel over the flattened (B*S) dimension — each of the 8
cores computes 1024 rows x all 12288 output columns.  Unlike the
column-parallel split this does not replicate the LoRA down-projection
(which is ~25% of the base GEMM's FLOPs), and the per-batch adapter mask
becomes a single per-core [128] vector (each core's rows live in one
batch).  Each core holds x^T for its rows resident in SBUF and streams W.

Device layout (per core, all matmuls bf16 with fp32 PSUM accumulation):
    xk  [128, 32, 1024]    x^T tiles: [k%128, k//128, m]
    wk  [96, 128, 32, 128] W^T tiles per output tile: [ot, k%128, k//128, o]
    at  [128, 3, 32, 128]  lora_A^T tiles: [k%128, p, k//128, nr]
    bt  [3, 128, 4096]     lora_B^T: [p, nr, o]   (nr = n*16 + r)
    bias[128, 96]          bias[ot*128+op] at [op, ot]
    wv  [128, 1]           scaling[n]*mask[n, batch(core)] at [n*16+r]
    out [96, 128, 1024]    out^T tiles: [ot, o, m]

Per output tile ot (96): 32 k-tile matmuls accumulate W^T x into PSUM
[o=128, m=1024]; one extra matmul per 512-wide m chunk accumulates the
LoRA up-projection (contraction over nr=128) into the same PSUM group; a
DVE tensor_scalar add applies bias while copying PSUM -> SBUF; DMA out.

Measured (8x axon trn2, loop-scaled timing so dispatch overhead cancels):
the chip flips between two discrete states — fast ~1.72 ms/iter and slow
~3.02 ms/iter (x1.76) — on a minutes timescale; the same NEFF measures
either value depending on when it runs.  In the fast state this schedule
runs at ~268 ns per N=512 matmul, matching the pure-PE streaming rate
measured by a DMA-free microbenchmark in the same state (no in-kernel
stalls: removing the whole W stream, the x reload, the output DMA, or the
DVE bias-add individually moves per-iter time <1%).  Matmul cost is
purely proportional to moving columns (N=256 costs exactly half of
N=512; stationary reuse is free), so the 6528x512-col schedule is at the
achievable floor.  fp8 was evaluated and rejected: e4m3 quantization
alone is 2.7-3.8% rel err vs the 2e-2 budget, and residual-correction
schemes cost more columns than bf16.  bf16 end-to-end relative error vs
the fp32 reference: ~2.0e-3.
"""

import numpy as np
import ml_dtypes
from contextlib import ExitStack

import concourse.bass as bass
import concourse.tile as tile
from concourse import bacc, mybir
from concourse.bass_utils import run_bass_kernel_spmd

BF16 = ml_dtypes.bfloat16

B, S, D = 4, 2048, 4096
OUT = 3 * D
N_CORES = 8
M = B * S                 # 8192 flattened rows
MC = M // N_CORES         # 1024 rows per core
P = 128
KT = D // P               # 32 k-tiles
OT = OUT // P             # 96 output tiles
OTP = OT // 3             # 32 output tiles per q/k/v block
NADP, R = 8, 16
NR = NADP * R             # 128 = contraction size of the up-projection
MM_N = 512                # moving-operand width per matmul
N_MCHUNK = MC // MM_N     # 2

_CACHE: dict = {}


def _build(loop_iters: int | None = None):
    """Trace + compile the per-core Bass program (same program on all cores).

    loop_iters: if set, wrap the body in a hardware For loop that executes
    it that many times per dispatch (used only for benchmarking)."""
    fp32 = mybir.dt.float32
    bf16 = mybir.dt.bfloat16

    nc = bacc.Bacc("TRN2", target_bir_lowering=False, debug=False,
                   num_devices=N_CORES)
    xk = nc.dram_tensor("xk", [P, KT, MC], bf16, kind="ExternalInput").ap()
    wk = nc.dram_tensor("wk", [OT, P, KT, P], bf16, kind="ExternalInput").ap()
    at = nc.dram_tensor("at", [P, 3, KT, NR], bf16, kind="ExternalInput").ap()
    bt = nc.dram_tensor("bt", [3, NR, D], bf16, kind="ExternalInput").ap()
    bias = nc.dram_tensor("bias", [P, OT], fp32, kind="ExternalInput").ap()
    wv = nc.dram_tensor("wv", [P, 1], fp32, kind="ExternalInput").ap()
    out = nc.dram_tensor("out", [OT, P, MC], fp32, kind="ExternalOutput").ap()

    with tile.TileContext(nc) as tc, ExitStack() as ctx:
        const = ctx.enter_context(tc.tile_pool(name="const", bufs=1))
        wpool = ctx.enter_context(tc.tile_pool(name="wpool", bufs=6))
        btpool = ctx.enter_context(tc.tile_pool(name="btpool", bufs=2))
        opool = ctx.enter_context(tc.tile_pool(name="opool", bufs=4))
        dppool = ctx.enter_context(tc.tile_pool(name="dppool", bufs=2, space="PSUM"))
        pspool = ctx.enter_context(tc.tile_pool(name="pspool", bufs=2, space="PSUM"))

        loop_cm = tc.For_i(0, loop_iters, 1) if loop_iters else None
        if loop_cm is not None:
            loop_cm.__enter__()
        try:
            # Resident inputs.  x is split per k-tile so the loads spread
            # across DMA queues; gpsimd (SWDGE) keeps the sync HWDGE ring
            # free for the W stream.
            xsb = const.tile([P, KT, MC], bf16, name="xsb")
            for kt in range(KT):
                nc.gpsimd.dma_start(xsb[:, kt, :], xk[:, kt, :])
            asb = const.tile([P, 3, KT, NR], bf16, name="asb")
            nc.gpsimd.dma_start(asb, at)
            biassb = const.tile([P, OT], fp32, name="biassb")
            nc.gpsimd.dma_start(biassb, bias)
            wvsb = const.tile([P, 1], fp32, name="wvsb")
            nc.gpsimd.dma_start(wvsb, wv)
            # Scaled down-projection result (x A^T * wv)^T, bf16: [nr, p, m]
            tmpsb = const.tile([P, 3, MC], bf16, name="tmpsb")

            # LoRA down-projection: tmp^T[nr, m] = A_p^T.T @ x^T per p/chunk.
            for p in range(3):
                for mc_i in range(N_MCHUNK):
                    msl = slice(mc_i * MM_N, (mc_i + 1) * MM_N)
                    dp = dppool.tile([P, MM_N], fp32, name="dp")
                    for kt in range(KT):
                        nc.tensor.matmul(dp, lhsT=asb[:, p, kt, :],
                                         rhs=xsb[:, kt, msl],
                                         start=(kt == 0), stop=(kt == KT - 1))
                    # scale by the per-partition adapter weight while
                    # copying PSUM -> SBUF
                    nc.scalar.mul(tmpsb[:, p, msl], dp, wvsb)

            # Main loop: 96 output tiles of [o=128, m=1024].
            for p in range(3):
                btsb = btpool.tile([NR, D], bf16, name="btsb")
                for jj in range(4):
                    osl = slice(jj * (D // 4), (jj + 1) * (D // 4))
                    nc.gpsimd.dma_start(btsb[:, osl], bt[p, :, osl])
                for j in range(OTP):
                    ot = p * OTP + j
                    wsb = wpool.tile([P, KT, P], bf16, name="wsb")
                    for h in range(4):
                        ksl = slice(h * (KT // 4), (h + 1) * (KT // 4))
                        nc.sync.dma_start(wsb[:, ksl, :], wk[ot, :, ksl, :])
                    ps = pspool.tile([P, MC], fp32, name="ps")
                    for kt in range(KT):
                        for mc_i in range(N_MCHUNK):
                            msl = slice(mc_i * MM_N, (mc_i + 1) * MM_N)
                            nc.tensor.matmul(ps[:, msl], lhsT=wsb[:, kt, :],
                                             rhs=xsb[:, kt, msl],
                                             start=(kt == 0), stop=False)
                    for mc_i in range(N_MCHUNK):
                        msl = slice(mc_i * MM_N, (mc_i + 1) * MM_N)
                        nc.tensor.matmul(ps[:, msl],
                                         lhsT=btsb[:, j * P:(j + 1) * P],
                                         rhs=tmpsb[:, p, msl],
                                         start=False, stop=True)
                    osb = opool.tile([P, MC], fp32, name="osb")
                    nc.vector.tensor_scalar_add(osb, ps, biassb[:, ot:ot + 1])
                    nc.scalar.dma_start(out[ot], osb)
        finally:
            if loop_cm is not None:
                loop_cm.__exit__(None, None, None)

    nc.compile()
    return nc


def get_nc(loop_iters: int | None = None):
    key = ("nc", loop_iters)
    if key not in _CACHE:
        _CACHE[key] = _build(loop_iters)
    return _CACHE[key]


def prep_in_maps(inputs: dict) -> list[dict]:
    """Shard + retile the full inputs into the 8 per-core input maps."""
    x = np.asarray(inputs["x"], np.float32).reshape(M, D)
    w = np.asarray(inputs["weight"], np.float32)
    bias = np.asarray(inputs["bias"], np.float32)
    lora_A = np.asarray(inputs["lora_A"], np.float32)
    lora_B = np.asarray(inputs["lora_B"], np.float32)
    scaling = np.asarray(inputs["scaling"], np.float32)
    masks = np.asarray(inputs["lora_masks"], np.float32)

    wk = np.ascontiguousarray(
        w.reshape(OT, P, KT, P).transpose(0, 3, 2, 1)).astype(BF16)
    at = np.ascontiguousarray(
        lora_A.reshape(3, NR, KT, P).transpose(3, 0, 2, 1)).astype(BF16)
    bt = np.ascontiguousarray(
        lora_B.transpose(0, 1, 3, 2).reshape(3, NR, D)).astype(BF16)
    biasd = np.ascontiguousarray(bias.reshape(OT, P).T)
    wmat = scaling[:, None] * masks          # [n, b]

    in_maps = []
    for c in range(N_CORES):
        xs = x[c * MC:(c + 1) * MC]          # [MC, D]
        xkc = np.ascontiguousarray(
            xs.reshape(MC, KT, P).transpose(2, 1, 0)).astype(BF16)
        b_idx = (c * MC) // S                # batch of this core's rows
        wvc = np.repeat(wmat[:, b_idx], R).astype(np.float32).reshape(P, 1)
        in_maps.append({"xk": xkc, "wk": wk, "at": at, "bt": bt,
                        "bias": biasd, "wv": wvc})
    return in_maps


def run_device(in_maps: list[dict]):
    nc = get_nc()
    return run_bass_kernel_spmd(nc, in_maps, core_ids=list(range(N_CORES)))


def assemble(results: list[dict]) -> np.ndarray:
    big = np.empty((M, OUT), np.float32)
    for c in range(N_CORES):
        big[c * MC:(c + 1) * MC] = results[c]["out"].reshape(OUT, MC).T
    return big.reshape(B, S, OUT)


def kernel(**inputs) -> np.ndarray:
    in_maps = prep_in_maps(inputs)
    res = run_device(in_maps)
    return assemble(res.results)



# revision 3
# speedup vs baseline: 1.0324x; 1.0324x over previous
"""Trainium2 Bass kernel for fused QKV linear + multi-adapter LoRA (moe_routing).

Reference computation (all fp32):
    base = x @ W^T + bias                      x:[B,S,D]  W:[3D,D]
    tmp[p,n,b,s,r]  = x . lora_A[p,n,r,:]      (down-projection, rank 16)
    tmp *= scaling[n] * lora_masks[n,b]
    lora[p,b,s,o]   = tmp . lora_B[p,n,o,r]    (up-projection, summed over n)
    out = base + concat_p(lora)                [B,S,3D]

Sharding: row-parallel over the flattened (B*S) dimension — each of the 8
cores computes 1024 rows x all 12288 output columns.  Unlike the
column-parallel split this does not replicate the LoRA down-projection
(which is ~25% of the base GEMM's FLOPs), and the per-batch adapter mask
becomes a single per-core [128] vector (each core's rows live in one
batch).  Each core holds x^T for its rows resident in SBUF and streams W.

Device layout (per core, all matmuls bf16 with fp32 PSUM accumulation):
    xk  [128, 32, 1024]    x^T tiles: [k%128, k//128, m]
    wk  [96, 128, 32, 128] W^T tiles per output tile: [ot, k%128, k//128, o]
    at  [128, 3, 32, 128]  lora_A^T tiles: [k%128, p, k//128, nr]
    bt  [3, 128, 4096]     lora_B^T: [p, nr, o]   (nr = n*16 + r)
    bias[128, 96]          bias[ot*128+op] at [op, ot]
    wv  [128, 1]           scaling[n]*mask[n, batch(core)] at [n*16+r]
    out [96, 128, 1024]    out^T tiles: [ot, o, m]

Per output tile ot (96): 32 k-tile matmuls accumulate W^T x into PSUM
[o=128, m=1024]; one extra matmul per 512-wide m chunk accumulates the
LoRA up-projection (contraction over nr=128) into the same PSUM group; a
DVE tensor_scalar add applies bias while copying PSUM -> SBUF; DMA out.

Measured (8x axon trn2, loop-scaled timing so dispatch overhead cancels):
the chip flips between two discrete states — fast ~1.72 ms/iter and slow
~3.02 ms/iter (x1.76) — on a minutes timescale; the same NEFF measures
either value depending on when it runs.  In the fast state this schedule
runs at ~268 ns per N=512 matmul, matching the pure-PE streaming rate
measured by a DMA-free microbenchmark in the same state (no in-kernel
stalls: removing the whole W stream, the x reload, the output DMA, or the
DVE bias-add individually moves per-iter time <1%; 10 interleaved
same-state rounds of {full kernel, kernel with ALL non-PE work removed,
flat 6528-matmul stream} are statistically identical).  Matmul cost is
purely proportional to moving columns (N=256 costs exactly half of
N=512; stationary reuse is free), so the 6528x512-col schedule is at the
achievable floor.  fp8 was evaluated and rejected: e4m3 quantization
alone is 2.7-3.8% rel err vs the 2e-2 budget, and residual-correction
schemes cost more columns than bf16.  bf16 end-to-end relative error vs
the fp32 reference: ~2.0e-3.
"""

import numpy as np
import ml_dtypes
from contextlib import ExitStack

import concourse.bass as bass
import concourse.tile as tile
from concourse import bacc, mybir
from concourse.bass_utils import run_bass_kernel_spmd

BF16 = ml_dtypes.bfloat16

B, S, D = 4, 2048, 4096
OUT = 3 * D
N_CORES = 8
M = B * S                 # 8192 flattened rows
MC = M // N_CORES         # 1024 rows per core
P = 128
KT = D // P               # 32 k-tiles
OT = OUT // P             # 96 output tiles
OTP = OT // 3             # 32 output tiles per q/k/v block
NADP, R = 8, 16
NR = NADP * R             # 128 = contraction size of the up-projection
MM_N = 512                # moving-operand width per matmul
N_MCHUNK = MC // MM_N     # 2

_CACHE: dict = {}


def _build(loop_iters: int | None = None):
    """Trace + compile the per-core Bass program (same program on all cores).

    loop_iters: if set, wrap the body in a hardware For loop that executes
    it that many times per dispatch (used only for benchmarking)."""
    fp32 = mybir.dt.float32
    bf16 = mybir.dt.bfloat16

    nc = bacc.Bacc("TRN2", target_bir_lowering=False, debug=False,
                   num_devices=N_CORES)
    xk = nc.dram_tensor("xk", [P, KT, MC], bf16, kind="ExternalInput").ap()
    wk = nc.dram_tensor("wk", [OT, P, KT, P], bf16, kind="ExternalInput").ap()
    at = nc.dram_tensor("at", [P, 3, KT, NR], bf16, kind="ExternalInput").ap()
    bt = nc.dram_tensor("bt", [3, NR, D], bf16, kind="ExternalInput").ap()
    bias = nc.dram_tensor("bias", [P, OT], fp32, kind="ExternalInput").ap()
    wv = nc.dram_tensor("wv", [P, 1], fp32, kind="ExternalInput").ap()
    out = nc.dram_tensor("out", [OT, P, MC], fp32, kind="ExternalOutput").ap()

    with tile.TileContext(nc) as tc, ExitStack() as ctx:
        const = ctx.enter_context(tc.tile_pool(name="const", bufs=1))
        wpool = ctx.enter_context(tc.tile_pool(name="wpool", bufs=6))
        btpool = ctx.enter_context(tc.tile_pool(name="btpool", bufs=2))
        opool = ctx.enter_context(tc.tile_pool(name="opool", bufs=4))
        dppool = ctx.enter_context(tc.tile_pool(name="dppool", bufs=2, space="PSUM"))
        pspool = ctx.enter_context(tc.tile_pool(name="pspool", bufs=2, space="PSUM"))

        loop_cm = tc.For_i(0, loop_iters, 1) if loop_iters else None
        if loop_cm is not None:
            loop_cm.__enter__()
        try:
            # Resident inputs.  x is split per k-tile so the loads spread
            # across DMA queues; gpsimd (SWDGE) keeps the sync HWDGE ring
            # free for the W stream.
            xsb = const.tile([P, KT, MC], bf16, name="xsb")
            for kt in range(KT):
                nc.gpsimd.dma_start(xsb[:, kt, :], xk[:, kt, :])
            asb = const.tile([P, 3, KT, NR], bf16, name="asb")
            nc.gpsimd.dma_start(asb, at)
            biassb = const.tile([P, OT], fp32, name="biassb")
            nc.gpsimd.dma_start(biassb, bias)
            wvsb = const.tile([P, 1], fp32, name="wvsb")
            nc.gpsimd.dma_start(wvsb, wv)
            # Scaled down-projection result (x A^T * wv)^T, bf16: [nr, p, m]
            tmpsb = const.tile([P, 3, MC], bf16, name="tmpsb")

            # LoRA down-projection: tmp^T[nr, m] = A_p^T.T @ x^T per p/chunk.
            for p in range(3):
                for mc_i in range(N_MCHUNK):
                    msl = slice(mc_i * MM_N, (mc_i + 1) * MM_N)
                    dp = dppool.tile([P, MM_N], fp32, name="dp")
                    for kt in range(KT):
                        nc.tensor.matmul(dp, lhsT=asb[:, p, kt, :],
                                         rhs=xsb[:, kt, msl],
                                         start=(kt == 0), stop=(kt == KT - 1))
                    # scale by the per-partition adapter weight while
                    # copying PSUM -> SBUF
                    nc.scalar.mul(tmpsb[:, p, msl], dp, wvsb)

            # Main loop: 96 output tiles of [o=128, m=1024].
            for p in range(3):
                btsb = btpool.tile([NR, D], bf16, name="btsb")
                for jj in range(4):
                    osl = slice(jj * (D // 4), (jj + 1) * (D // 4))
                    nc.gpsimd.dma_start(btsb[:, osl], bt[p, :, osl])
                for j in range(OTP):
                    ot = p * OTP + j
                    wsb = wpool.tile([P, KT, P], bf16, name="wsb")
                    for h in range(4):
                        ksl = slice(h * (KT // 4), (h + 1) * (KT // 4))
                        nc.sync.dma_start(wsb[:, ksl, :], wk[ot, :, ksl, :])
                    ps = pspool.tile([P, MC], fp32, name="ps")
                    for kt in range(KT):
                        for mc_i in range(N_MCHUNK):
                            msl = slice(mc_i * MM_N, (mc_i + 1) * MM_N)
                            nc.tensor.matmul(ps[:, msl], lhsT=wsb[:, kt, :],
                                             rhs=xsb[:, kt, msl],
                                             start=(kt == 0), stop=False)
                    for mc_i in range(N_MCHUNK):
                        msl = slice(mc_i * MM_N, (mc_i + 1) * MM_N)
                        nc.tensor.matmul(ps[:, msl],
                                         lhsT=btsb[:, j * P:(j + 1) * P],
                                         rhs=tmpsb[:, p, msl],
                                         start=False, stop=True)
                    osb = opool.tile([P, MC], fp32, name="osb")
                    nc.vector.tensor_scalar_add(osb, ps, biassb[:, ot:ot + 1])
                    nc.scalar.dma_start(out[ot], osb)
        finally:
            if loop_cm is not None:
                loop_cm.__exit__(None, None, None)

    nc.compile()
    return nc


def get_nc(loop_iters: int | None = None):
    key = ("nc", loop_iters)
    if key not in _CACHE:
        _CACHE[key] = _build(loop_iters)
    return _CACHE[key]


def prep_in_maps(inputs: dict) -> list[dict]:
    """Shard + retile the full inputs into the 8 per-core input maps."""
    x = np.asarray(inputs["x"], np.float32).reshape(M, D)
    w = np.asarray(inputs["weight"], np.float32)
    bias = np.asarray(inputs["bias"], np.float32)
    lora_A = np.asarray(inputs["lora_A"], np.float32)
    lora_B = np.asarray(inputs["lora_B"], np.float32)
    scaling = np.asarray(inputs["scaling"], np.float32)
    masks = np.asarray(inputs["lora_masks"], np.float32)

    wk = np.ascontiguousarray(
        w.reshape(OT, P, KT, P).transpose(0, 3, 2, 1)).astype(BF16)
    at = np.ascontiguousarray(
        lora_A.reshape(3, NR, KT, P).transpose(3, 0, 2, 1)).astype(BF16)
    bt = np.ascontiguousarray(
        lora_B.transpose(0, 1, 3, 2).reshape(3, NR, D)).astype(BF16)
    biasd = np.ascontiguousarray(bias.reshape(OT, P).T)
    wmat = scaling[:, None] * masks          # [n, b]

    in_maps = []
    for c in range(N_CORES):
        xs = x[c * MC:(c + 1) * MC]          # [MC, D]
        xkc = np.ascontiguousarray(
            xs.reshape(MC, KT, P).transpose(2, 1, 0)).astype(BF16)
        b_idx = (c * MC) // S                # batch of this core's rows
        wvc = np.repeat(wmat[:, b_idx], R).astype(np.float32).reshape(P, 1)
        in_maps.append({"xk": xkc, "wk": wk, "at": at, "bt": bt,
                        "bias": biasd, "wv": wvc})
    return in_maps


def run_device(in_maps: list[dict]):
    nc = get_nc()
    return run_bass_kernel_spmd(nc, in_maps, core_ids=list(range(N_CORES)))


def assemble(results: list[dict]) -> np.ndarray:
    big = np.empty((M, OUT), np.float32)
    for c in range(N_CORES):
        big[c * MC:(c + 1) * MC] = results[c]["out"].reshape(OUT, MC).T
    return big.reshape(B, S, OUT)


def kernel(**inputs) -> np.ndarray:
    in_maps = prep_in_maps(inputs)
    res = run_device(in_maps)
    return assemble(res.results)



# revision 4
# speedup vs baseline: 1.1150x; 1.0799x over previous
"""Trainium2 Bass kernel for fused QKV linear + multi-adapter LoRA (moe_routing).

Reference computation (all fp32):
    base = x @ W^T + bias                      x:[B,S,D]  W:[3D,D]
    tmp[p,n,b,s,r]  = x . lora_A[p,n,r,:]      (down-projection, rank 16)
    tmp *= scaling[n] * lora_masks[n,b]
    lora[p,b,s,o]   = tmp . lora_B[p,n,o,r]    (up-projection, summed over n)
    out = base + concat_p(lora)                [B,S,3D]

Sharding: row-parallel over the flattened (B*S) dimension — each of the 8
cores computes 1024 rows x all 12288 output columns.  Unlike the
column-parallel split this does not replicate the LoRA down-projection
(which is ~25% of the base GEMM's FLOPs), and the per-batch adapter mask
becomes a single per-core [128] vector (each core's rows live in one
batch).  Each core holds x^T for its rows resident in SBUF and streams W.

Mixed precision: k-tiles 0..25 of the 32-tile contraction run in bf16;
k-tiles 26..31 run in fp8 e4m3 with DoubleRow perf mode (double-pumped PE,
~1.8x per covered k-tile, measured 293 ns per [128,2,512] instruction vs
534 ns bf16-equivalent).  Operand scales x*0.5 / W*2 make the fp8 product
scale 1, so fp8 instructions accumulate directly into the same PSUM group
as the bf16 ones — no recombine pass.  Measured end-to-end relative error
vs the fp32 reference: ~1.4e-2 (budget 2e-2; pure bf16 was 2.0e-3, the
fp8 quantization of 6/32 of the contraction dominates at sqrt(6/32)*3.8%).

Device layout (per core, fp32 PSUM accumulation):
    xk  [128, 26, 1024]     x^T bf16 tiles: [k%128, k//128, m], k-tiles 0..25
    xk8 [128, 6, 1024]      x^T*0.5 fp8 tiles, k-tiles 26..31
    wk  [96, 128, 26, 128]  W^T bf16 tiles per output tile
    wk8 [96, 128, 6, 128]   W^T*2 fp8 tiles
    at  [128, 3, 26, 128]   lora_A^T bf16 tiles: [k%128, p, k//128, nr]
    at8 [128, 3, 6, 128]    lora_A^T*2 fp8 tiles
    bt  [3, 128, 4096]      lora_B^T: [p, nr, o]   (nr = n*16 + r)
    bias[128, 96]           bias[ot*128+op] at [op, ot]
    wv  [128, 1]            scaling[n]*mask[n, batch(core)] at [n*16+r]
    out [96, 128, 1024]     out^T tiles: [ot, o, m]

Per output tile ot (96): 52 bf16 + 6 fp8-DoubleRow matmuls accumulate
W^T x into PSUM [o=128, m=1024]; one extra matmul per 512-wide m chunk
accumulates the LoRA up-projection (contraction over nr=128) into the same
PSUM group; a DVE tensor_scalar add applies bias while copying PSUM ->
SBUF; DMA out.  The LoRA down-projection uses the same bf16/fp8 k split.

Performance notes (8x axon trn2, loop-scaled timing so dispatch overhead
cancels): the chip flips between two discrete states — fast and ~1.76x
slower — on a minutes timescale.  In the fast state the all-bf16 version
of this schedule measured ~1.72 ms/iter, statistically identical to a
flat 6528-matmul stream and to the kernel with ALL non-PE work removed
(DMA/DVE fully hidden; cost is purely proportional to moving columns;
1 busy core sustains 2.32 GHz, 8 busy cores ~1.92 GHz each — chip-level
DVFS).  The fp8 split removes ~8% of PE column-time from that floor.
"""

import numpy as np
import ml_dtypes
from contextlib import ExitStack

import concourse.bass as bass
import concourse.tile as tile
from concourse import bacc, mybir
from concourse.bass_utils import run_bass_kernel_spmd

BF16 = ml_dtypes.bfloat16
F8 = ml_dtypes.float8_e4m3

B, S, D = 4, 2048, 4096
OUT = 3 * D
N_CORES = 8
M = B * S                 # 8192 flattened rows
MC = M // N_CORES         # 1024 rows per core
P = 128
KT = D // P               # 32 k-tiles
FP8_KT = 6                # k-tiles 26..31 run in fp8 DoubleRow
BF_KT = KT - FP8_KT       # 26 bf16 k-tiles
OT = OUT // P             # 96 output tiles
OTP = OT // 3             # 32 output tiles per q/k/v block
NADP, R = 8, 16
NR = NADP * R             # 128 = contraction size of the up-projection
MM_N = 512                # moving-operand width per matmul
N_MCHUNK = MC // MM_N     # 2
SX, SW = 0.5, 2.0         # fp8 operand scales; SX*SW == 1

_CACHE: dict = {}


def _build(loop_iters: int | None = None):
    """Trace + compile the per-core Bass program (same program on all cores).

    loop_iters: if set, wrap the body in a hardware For loop that executes
    it that many times per dispatch (used only for benchmarking)."""
    fp32 = mybir.dt.float32
    bf16 = mybir.dt.bfloat16
    f8 = mybir.dt.float8e4
    DR = mybir.MatmulPerfMode.DoubleRow

    nc = bacc.Bacc("TRN2", target_bir_lowering=False, debug=False,
                   num_devices=N_CORES)
    xk = nc.dram_tensor("xk", [P, BF_KT, MC], bf16, kind="ExternalInput").ap()
    xk8 = nc.dram_tensor("xk8", [P, FP8_KT, MC], f8, kind="ExternalInput").ap()
    wk = nc.dram_tensor("wk", [OT, P, BF_KT, P], bf16, kind="ExternalInput").ap()
    wk8 = nc.dram_tensor("wk8", [OT, P, FP8_KT, P], f8, kind="ExternalInput").ap()
    at = nc.dram_tensor("at", [P, 3, BF_KT, NR], bf16, kind="ExternalInput").ap()
    at8 = nc.dram_tensor("at8", [P, 3, FP8_KT, NR], f8, kind="ExternalInput").ap()
    bt = nc.dram_tensor("bt", [3, NR, D], bf16, kind="ExternalInput").ap()
    bias = nc.dram_tensor("bias", [P, OT], fp32, kind="ExternalInput").ap()
    wv = nc.dram_tensor("wv", [P, 1], fp32, kind="ExternalInput").ap()
    out = nc.dram_tensor("out", [OT, P, MC], fp32, kind="ExternalOutput").ap()

    with tile.TileContext(nc) as tc, ExitStack() as ctx:
        const = ctx.enter_context(tc.tile_pool(name="const", bufs=1))
        wpool = ctx.enter_context(tc.tile_pool(name="wpool", bufs=6))
        w8pool = ctx.enter_context(tc.tile_pool(name="w8pool", bufs=6))
        btpool = ctx.enter_context(tc.tile_pool(name="btpool", bufs=2))
        opool = ctx.enter_context(tc.tile_pool(name="opool", bufs=4))
        dppool = ctx.enter_context(tc.tile_pool(name="dppool", bufs=2, space="PSUM"))
        pspool = ctx.enter_context(tc.tile_pool(name="pspool", bufs=2, space="PSUM"))

        loop_cm = tc.For_i(0, loop_iters, 1) if loop_iters else None
        if loop_cm is not None:
            loop_cm.__enter__()
        try:
            # Resident inputs.  x is split per k-tile so the loads spread
            # across DMA queues; gpsimd (SWDGE) keeps the sync HWDGE ring
            # free for the W stream.
            xsb = const.tile([P, BF_KT, MC], bf16, name="xsb")
            for kt in range(BF_KT):
                nc.gpsimd.dma_start(xsb[:, kt, :], xk[:, kt, :])
            xsb8 = const.tile([P, FP8_KT, MC], f8, name="xsb8")
            nc.gpsimd.dma_start(xsb8, xk8)
            asb = const.tile([P, 3, BF_KT, NR], bf16, name="asb")
            nc.gpsimd.dma_start(asb, at)
            asb8 = const.tile([P, 3, FP8_KT, NR], f8, name="asb8")
            nc.gpsimd.dma_start(asb8, at8)
            biassb = const.tile([P, OT], fp32, name="biassb")
            nc.gpsimd.dma_start(biassb, bias)
            wvsb = const.tile([P, 1], fp32, name="wvsb")
            nc.gpsimd.dma_start(wvsb, wv)
            # Scaled down-projection result (x A^T * wv)^T, bf16: [nr, p, m]
            tmpsb = const.tile([P, 3, MC], bf16, name="tmpsb")

            # LoRA down-projection: tmp^T[nr, m] = A_p^T.T @ x^T per p/chunk,
            # bf16 k-tiles 0..25 then fp8 DoubleRow k-tiles 26..31.
            for p in range(3):
                for mc_i in range(N_MCHUNK):
                    msl = slice(mc_i * MM_N, (mc_i + 1) * MM_N)
                    dp = dppool.tile([P, MM_N], fp32, name="dp")
                    for kt in range(BF_KT):
                        nc.tensor.matmul(dp, lhsT=asb[:, p, kt, :],
                                         rhs=xsb[:, kt, msl],
                                         start=(kt == 0), stop=False)
                    for t in range(FP8_KT // 2):
                        ks = slice(2 * t, 2 * t + 2)
                        nc.tensor.matmul(dp, lhsT=asb8[:, p, ks, :],
                                         rhs=xsb8[:, ks, msl],
                                         start=False, stop=(t == FP8_KT // 2 - 1),
                                         perf_mode=DR)
                    # scale by the per-partition adapter weight while
                    # copying PSUM -> SBUF
                    nc.scalar.mul(tmpsb[:, p, msl], dp, wvsb)

            # Main loop: 96 output tiles of [o=128, m=1024].
            for p in range(3):
                btsb = btpool.tile([NR, D], bf16, name="btsb")
                for jj in range(4):
                    osl = slice(jj * (D // 4), (jj + 1) * (D // 4))
                    nc.gpsimd.dma_start(btsb[:, osl], bt[p, :, osl])
                for j in range(OTP):
                    ot = p * OTP + j
                    wsb = wpool.tile([P, BF_KT, P], bf16, name="wsb")
                    for h in range(2):
                        ksl = slice(h * (BF_KT // 2), (h + 1) * (BF_KT // 2))
                        nc.sync.dma_start(wsb[:, ksl, :], wk[ot, :, ksl, :])
                    wsb8 = w8pool.tile([P, FP8_KT, P], f8, name="wsb8")
                    nc.sync.dma_start(wsb8, wk8[ot])
                    ps = pspool.tile([P, MC], fp32, name="ps")
                    for kt in range(BF_KT):
                        for mc_i in range(N_MCHUNK):
                            msl = slice(mc_i * MM_N, (mc_i + 1) * MM_N)
                            nc.tensor.matmul(ps[:, msl], lhsT=wsb[:, kt, :],
                                             rhs=xsb[:, kt, msl],
                                             start=(kt == 0), stop=False)
                    for t in range(FP8_KT // 2):
                        ks = slice(2 * t, 2 * t + 2)
                        for mc_i in range(N_MCHUNK):
                            msl = slice(mc_i * MM_N, (mc_i + 1) * MM_N)
                            nc.tensor.matmul(ps[:, msl], lhsT=wsb8[:, ks, :],
                                             rhs=xsb8[:, ks, msl],
                                             start=False, stop=False,
                                             perf_mode=DR)
                    for mc_i in range(N_MCHUNK):
                        msl = slice(mc_i * MM_N, (mc_i + 1) * MM_N)
                        nc.tensor.matmul(ps[:, msl],
                                         lhsT=btsb[:, j * P:(j + 1) * P],
                                         rhs=tmpsb[:, p, msl],
                                         start=False, stop=True)
                    osb = opool.tile([P, MC], fp32, name="osb")
                    nc.vector.tensor_scalar_add(osb, ps, biassb[:, ot:ot + 1])
                    nc.scalar.dma_start(out[ot], osb)
        finally:
            if loop_cm is not None:
                loop_cm.__exit__(None, None, None)

    nc.compile()
    return nc


def get_nc(loop_iters: int | None = None):
    key = ("nc", loop_iters)
    if key not in _CACHE:
        _CACHE[key] = _build(loop_iters)
    return _CACHE[key]


def prep_in_maps(inputs: dict) -> list[dict]:
    """Shard + retile the full inputs into the 8 per-core input maps."""
    x = np.asarray(inputs["x"], np.float32).reshape(M, D)
    w = np.asarray(inputs["weight"], np.float32)
    bias = np.asarray(inputs["bias"], np.float32)
    lora_A = np.asarray(inputs["lora_A"], np.float32)
    lora_B = np.asarray(inputs["lora_B"], np.float32)
    scaling = np.asarray(inputs["scaling"], np.float32)
    masks = np.asarray(inputs["lora_masks"], np.float32)

    wt = w.reshape(OT, P, KT, P).transpose(0, 3, 2, 1)    # [ot, pk, kt, po]
    wk = np.ascontiguousarray(wt[:, :, :BF_KT]).astype(BF16)
    wk8 = np.ascontiguousarray(wt[:, :, BF_KT:] * SW).astype(F8)
    att = lora_A.reshape(3, NR, KT, P).transpose(3, 0, 2, 1)  # [pk, p, kt, nr]
    at = np.ascontiguousarray(att[:, :, :BF_KT]).astype(BF16)
    at8 = np.ascontiguousarray(att[:, :, BF_KT:] * SW).astype(F8)
    bt = np.ascontiguousarray(
        lora_B.transpose(0, 1, 3, 2).reshape(3, NR, D)).astype(BF16)
    biasd = np.ascontiguousarray(bias.reshape(OT, P).T)
    wmat = scaling[:, None] * masks          # [n, b]

    in_maps = []
    for c in range(N_CORES):
        xs = x[c * MC:(c + 1) * MC]          # [MC, D]
        xt = xs.reshape(MC, KT, P).transpose(2, 1, 0)     # [pk, kt, m]
        xkc = np.ascontiguousarray(xt[:, :BF_KT]).astype(BF16)
        xk8c = np.ascontiguousarray(xt[:, BF_KT:] * SX).astype(F8)
        b_idx = (c * MC) // S                # batch of this core's rows
        wvc = np.repeat(wmat[:, b_idx], R).astype(np.float32).reshape(P, 1)
        in_maps.append({"xk": xkc, "xk8": xk8c, "wk": wk, "wk8": wk8,
                        "at": at, "at8": at8, "bt": bt,
                        "bias": biasd, "wv": wvc})
    return in_maps


def run_device(in_maps: list[dict]):
    nc = get_nc()
    return run_bass_kernel_spmd(nc, in_maps, core_ids=list(range(N_CORES)))


def assemble(results: list[dict]) -> np.ndarray:
    big = np.empty((M, OUT), np.float32)
    for c in range(N_CORES):
        big[c * MC:(c + 1) * MC] = results[c]["out"].reshape(OUT, MC).T
    return big.reshape(B, S, OUT)


def kernel(**inputs) -> np.ndarray:
    in_maps = prep_in_maps(inputs)
    res = run_device(in_maps)
    return assemble(res.results)


# revision 7
# speedup vs baseline: 1.2907x; 1.1576x over previous
"""Trainium2 Bass kernel for fused QKV linear + multi-adapter LoRA (moe_routing).

Reference computation (all fp32):
    base = x @ W^T + bias                      x:[B,S,D]  W:[3D,D]
    tmp[p,n,b,s,r]  = x . lora_A[p,n,r,:]      (down-projection, rank 16)
    tmp *= scaling[n] * lora_masks[n,b]
    lora[p,b,s,o]   = tmp . lora_B[p,n,o,r]    (up-projection, summed over n)
    out = base + concat_p(lora)                [B,S,3D]

Sharding: row-parallel over the flattened (B*S) dimension — each of the 8
cores computes 1024 rows x all 12288 output columns.  Unlike the
column-parallel split this does not replicate the LoRA down-projection
(which is ~25% of the base GEMM's FLOPs), and the per-batch adapter mask
becomes a single per-core [128] vector (each core's rows live in one
batch).  Each core holds x^T for its rows resident in SBUF and streams W.

Mixed precision: k-tiles 0..23 of the 32-tile contraction run in bf16;
k-tiles 24..31 run in fp8 e4m3 with DoubleRow perf mode (double-pumped PE,
~1.8x per covered k-tile, measured 293 ns per [128,2,512] instruction vs
534 ns bf16-equivalent).  Operand scales x*0.5 / W*2 make the fp8 product
scale 1, so fp8 instructions accumulate directly into the same PSUM group
as the bf16 ones — no recombine pass.  Measured end-to-end relative error
vs the fp32 reference: ~1.6e-2 (budget 2e-2; pure bf16 was 2.0e-3, the
fp8 quantization of 8/32 of the contraction dominates at sqrt(8/32)*3.8%).

Device layout (per core, fp32 PSUM accumulation):
    xk  [128, 24, 1024]     x^T bf16 tiles: [k%128, k//128, m], k-tiles 0..23
    xk8 [128, 8, 1024]      x^T*0.5 fp8 tiles, k-tiles 24..31
    wk  [96, 128, 24, 128]  W^T bf16 tiles per output tile
    wk8 [96, 128, 8, 128]   W^T*2 fp8 tiles
    at  [128, 3, 24, 128]   lora_A^T bf16 tiles: [k%128, p, k//128, nr]
    at8 [128, 3, 8, 128]    lora_A^T*2 fp8 tiles
    bt  [3, 128, 4096]      lora_B^T: [p, nr, o]   (nr = n*16 + r)
    bias[128, 96]           bias[ot*128+op] at [op, ot]
    wv  [128, 1]            scaling[n]*mask[n, batch(core)] at [n*16+r]
    out [96, 128, 1024]     out^T tiles: [ot, o, m]

Per output tile ot (96): 48 bf16 + 8 fp8-DoubleRow matmuls accumulate
W^T x into PSUM [o=128, m=1024]; one extra matmul per 512-wide m chunk
accumulates the LoRA up-projection (contraction over nr=128) into the same
PSUM group; a DVE tensor_scalar add applies bias while copying PSUM ->
SBUF; DMA out.  The LoRA down-projection uses the same bf16/fp8 k split.

Performance notes (8x axon trn2, loop-scaled timing so dispatch overhead
cancels): the chip flips between two discrete states — fast and ~1.76x
slower — on a minutes timescale.  In the fast state the all-bf16 version
of this schedule measured ~1.72 ms/iter, statistically identical to a
flat 6528-matmul stream and to the kernel with ALL non-PE work removed
(DMA/DVE fully hidden; cost is purely proportional to moving columns;
1 busy core sustains 2.32 GHz, 8 busy cores ~1.92 GHz each — chip-level
DVFS).  The fp8 split removes ~11% of PE column-time from that floor.
"""

import numpy as np
import ml_dtypes
from contextlib import ExitStack

import concourse.bass as bass
import concourse.tile as tile
from concourse import bacc, mybir
from concourse.bass_utils import run_bass_kernel_spmd

BF16 = ml_dtypes.bfloat16
F8 = ml_dtypes.float8_e4m3

B, S, D = 4, 2048, 4096
OUT = 3 * D
N_CORES = 8
M = B * S                 # 8192 flattened rows
MC = M // N_CORES         # 1024 rows per core
P = 128
KT = D // P               # 32 k-tiles
FP8_KT = 8                # k-tiles 24..31 run in fp8 DoubleRow
BF_KT = KT - FP8_KT       # 24 bf16 k-tiles
OT = OUT // P             # 96 output tiles
OTP = OT // 3             # 32 output tiles per q/k/v block
NADP, R = 8, 16
NR = NADP * R             # 128 = contraction size of the up-projection
MM_N = 512                # moving-operand width per matmul
N_MCHUNK = MC // MM_N     # 2
SX, SW = 0.5, 2.0         # fp8 operand scales; SX*SW == 1

_CACHE: dict = {}


def _build(loop_iters: int | None = None):
    """Trace + compile the per-core Bass program (same program on all cores).

    loop_iters: if set, wrap the body in a hardware For loop that executes
    it that many times per dispatch (used only for benchmarking)."""
    fp32 = mybir.dt.float32
    bf16 = mybir.dt.bfloat16
    f8 = mybir.dt.float8e4
    DR = mybir.MatmulPerfMode.DoubleRow

    nc = bacc.Bacc("TRN2", target_bir_lowering=False, debug=False,
                   num_devices=N_CORES)
    xk = nc.dram_tensor("xk", [P, BF_KT, MC], bf16, kind="ExternalInput").ap()
    xk8 = nc.dram_tensor("xk8", [P, FP8_KT, MC], f8, kind="ExternalInput").ap()
    wk = nc.dram_tensor("wk", [OT, P, BF_KT, P], bf16, kind="ExternalInput").ap()
    wk8 = nc.dram_tensor("wk8", [OT, P, FP8_KT, P], f8, kind="ExternalInput").ap()
    at = nc.dram_tensor("at", [P, 3, BF_KT, NR], bf16, kind="ExternalInput").ap()
    at8 = nc.dram_tensor("at8", [P, 3, FP8_KT, NR], f8, kind="ExternalInput").ap()
    bt = nc.dram_tensor("bt", [3, NR, D], bf16, kind="ExternalInput").ap()
    bias = nc.dram_tensor("bias", [P, OT], fp32, kind="ExternalInput").ap()
    wv = nc.dram_tensor("wv", [P, 1], fp32, kind="ExternalInput").ap()
    out = nc.dram_tensor("out", [OT, P, MC], fp32, kind="ExternalOutput").ap()

    with tile.TileContext(nc) as tc, ExitStack() as ctx:
        const = ctx.enter_context(tc.tile_pool(name="const", bufs=1))
        wpool = ctx.enter_context(tc.tile_pool(name="wpool", bufs=6))
        w8pool = ctx.enter_context(tc.tile_pool(name="w8pool", bufs=6))
        btpool = ctx.enter_context(tc.tile_pool(name="btpool", bufs=2))
        opool = ctx.enter_context(tc.tile_pool(name="opool", bufs=4))
        dppool = ctx.enter_context(tc.tile_pool(name="dppool", bufs=2, space="PSUM"))
        pspool = ctx.enter_context(tc.tile_pool(name="pspool", bufs=2, space="PSUM"))

        loop_cm = tc.For_i(0, loop_iters, 1) if loop_iters else None
        if loop_cm is not None:
            loop_cm.__enter__()
        try:
            # Resident inputs.  x is split per k-tile so the loads spread
            # across DMA queues; gpsimd (SWDGE) keeps the sync HWDGE ring
            # free for the W stream.
            xsb = const.tile([P, BF_KT, MC], bf16, name="xsb")
            for kt in range(BF_KT):
                nc.gpsimd.dma_start(xsb[:, kt, :], xk[:, kt, :])
            xsb8 = const.tile([P, FP8_KT, MC], f8, name="xsb8")
            nc.gpsimd.dma_start(xsb8, xk8)
            asb = const.tile([P, 3, BF_KT, NR], bf16, name="asb")
            nc.gpsimd.dma_start(asb, at)
            asb8 = const.tile([P, 3, FP8_KT, NR], f8, name="asb8")
            nc.gpsimd.dma_start(asb8, at8)
            biassb = const.tile([P, OT], fp32, name="biassb")
            nc.gpsimd.dma_start(biassb, bias)
            wvsb = const.tile([P, 1], fp32, name="wvsb")
            nc.gpsimd.dma_start(wvsb, wv)
            # Scaled down-projection result (x A^T * wv)^T, bf16: [nr, p, m]
            tmpsb = const.tile([P, 3, MC], bf16, name="tmpsb")

            # LoRA down-projection: tmp^T[nr, m] = A_p^T.T @ x^T per p/chunk,
            # bf16 k-tiles 0..23 then fp8 DoubleRow k-tiles 24..31.
            for p in range(3):
                for mc_i in range(N_MCHUNK):
                    msl = slice(mc_i * MM_N, (mc_i + 1) * MM_N)
                    dp = dppool.tile([P, MM_N], fp32, name="dp")
                    for kt in range(BF_KT):
                        nc.tensor.matmul(dp, lhsT=asb[:, p, kt, :],
                                         rhs=xsb[:, kt, msl],
                                         start=(kt == 0), stop=False)
                    for t in range(FP8_KT // 2):
                        ks = slice(2 * t, 2 * t + 2)
                        nc.tensor.matmul(dp, lhsT=asb8[:, p, ks, :],
                                         rhs=xsb8[:, ks, msl],
                                         start=False, stop=(t == FP8_KT // 2 - 1),
                                         perf_mode=DR)
                    # scale by the per-partition adapter weight while
                    # copying PSUM -> SBUF
                    nc.scalar.mul(tmpsb[:, p, msl], dp, wvsb)

            # Main loop: 96 output tiles of [o=128, m=1024].
            for p in range(3):
                btsb = btpool.tile([NR, D], bf16, name="btsb")
                for jj in range(4):
                    osl = slice(jj * (D // 4), (jj + 1) * (D // 4))
                    nc.gpsimd.dma_start(btsb[:, osl], bt[p, :, osl])
                for j in range(OTP):
                    ot = p * OTP + j
                    wsb = wpool.tile([P, BF_KT, P], bf16, name="wsb")
                    for h in range(2):
                        ksl = slice(h * (BF_KT // 2), (h + 1) * (BF_KT // 2))
                        nc.sync.dma_start(wsb[:, ksl, :], wk[ot, :, ksl, :])
                    wsb8 = w8pool.tile([P, FP8_KT, P], f8, name="wsb8")
                    nc.sync.dma_start(wsb8, wk8[ot])
                    ps = pspool.tile([P, MC], fp32, name="ps")
                    for kt in range(BF_KT):
                        for mc_i in range(N_MCHUNK):
                            msl = slice(mc_i * MM_N, (mc_i + 1) * MM_N)
                            nc.tensor.matmul(ps[:, msl], lhsT=wsb[:, kt, :],
                                             rhs=xsb[:, kt, msl],
                                             start=(kt == 0), stop=False)
                    for t in range(FP8_KT // 2):
                        ks = slice(2 * t, 2 * t + 2)
                        for mc_i in range(N_MCHUNK):
                            msl = slice(mc_i * MM_N, (mc_i + 1) * MM_N)
                            nc.tensor.matmul(ps[:, msl], lhsT=wsb8[:, ks, :],
                                             rhs=xsb8[:, ks, msl],
                                             start=False, stop=False,
                                             perf_mode=DR)
                    for mc_i in range(N_MCHUNK):
                        msl = slice(mc_i * MM_N, (mc_i + 1) * MM_N)
                        nc.tensor.matmul(ps[:, msl],
                                         lhsT=btsb[:, j * P:(j + 1) * P],
                                         rhs=tmpsb[:, p, msl],
                                         start=False, stop=True)
                    osb = opool.tile([P, MC], fp32, name="osb")
                    nc.vector.tensor_scalar_add(osb, ps, biassb[:, ot:ot + 1])
                    nc.scalar.dma_start(out[ot], osb)
        finally:
            if loop_cm is not None:
                loop_cm.__exit__(None, None, None)

    nc.compile()
    return nc


def get_nc(loop_iters: int | None = None):
    key = ("nc", loop_iters)
    if key not in _CACHE:
        _CACHE[key] = _build(loop_iters)
    return _CACHE[key]


def prep_in_maps(inputs: dict) -> list[dict]:
    """Shard + retile the full inputs into the 8 per-core input maps."""
    x = np.asarray(inputs["x"], np.float32).reshape(M, D)
    w = np.asarray(inputs["weight"], np.float32)
    bias = np.asarray(inputs["bias"], np.float32)
    lora_A = np.asarray(inputs["lora_A"], np.float32)
    lora_B = np.asarray(inputs["lora_B"], np.float32)
    scaling = np.asarray(inputs["scaling"], np.float32)
    masks = np.asarray(inputs["lora_masks"], np.float32)

    wt = w.reshape(OT, P, KT, P).transpose(0, 3, 2, 1)    # [ot, pk, kt, po]
    wk = np.ascontiguousarray(wt[:, :, :BF_KT]).astype(BF16)
    wk8 = np.ascontiguousarray(wt[:, :, BF_KT:] * SW).astype(F8)
    att = lora_A.reshape(3, NR, KT, P).transpose(3, 0, 2, 1)  # [pk, p, kt, nr]
    at = np.ascontiguousarray(att[:, :, :BF_KT]).astype(BF16)
    at8 = np.ascontiguousarray(att[:, :, BF_KT:] * SW).astype(F8)
    bt = np.ascontiguousarray(
        lora_B.transpose(0, 1, 3, 2).reshape(3, NR, D)).astype(BF16)
    biasd = np.ascontiguousarray(bias.reshape(OT, P).T)
    wmat = scaling[:, None] * masks          # [n, b]

    in_maps = []
    for c in range(N_CORES):
        xs = x[c * MC:(c + 1) * MC]          # [MC, D]
        xt = xs.reshape(MC, KT, P).transpose(2, 1, 0)     # [pk, kt, m]
        xkc = np.ascontiguousarray(xt[:, :BF_KT]).astype(BF16)
        xk8c = np.ascontiguousarray(xt[:, BF_KT:] * SX).astype(F8)
        b_idx = (c * MC) // S                # batch of this core's rows
        wvc = np.repeat(wmat[:, b_idx], R).astype(np.float32).reshape(P, 1)
        in_maps.append({"xk": xkc, "xk8": xk8c, "wk": wk, "wk8": wk8,
                        "at": at, "at8": at8, "bt": bt,
                        "bias": biasd, "wv": wvc})
    return in_maps


def run_device(in_maps: list[dict]):
    nc = get_nc()
    return run_bass_kernel_spmd(nc, in_maps, core_ids=list(range(N_CORES)))


def assemble(results: list[dict]) -> np.ndarray:
    big = np.empty((M, OUT), np.float32)
    for c in range(N_CORES):
        big[c * MC:(c + 1) * MC] = results[c]["out"].reshape(OUT, MC).T
    return big.reshape(B, S, OUT)


def kernel(**inputs) -> np.ndarray:
    in_maps = prep_in_maps(inputs)
    res = run_device(in_maps)
    return assemble(res.results)
